# revision 5
# baseline (speedup 1.0000x reference)
"""BasesDecomposition (R-GCN style) message passing kernel for Trainium2.

Single-phase design (8 NeuronCores, SPMD — one program, per-core data):
  - Nodes sharded by row: core c owns targets [c*nl, (c+1)*nl).
  - Edges symmetrized on host, owned by target core.
  - Per-relation W_r = sum_b rbw[r,b] bases[b] on host (bf16 on device).
  - Edges grouped into (source-bank, relation)-pure chunks of 128.
    Source banks of 25000 rows keep dma_gather indices within int16.
  - Per batch (<=16 chunks, bank-pure): one transposed bf16 dma_gather
    fetches x^T columns for all chunk edges; per chunk one matmul
    msg[e,o] = xT_chunk^T @ W_r (PSUM) and one ew-scaled copy to the
    message buffer (DVE/ACT alternating).
  - One f32 dma_scatter_add per batch adds message rows into a DRAM
    accumulator. Duplicate targets within one scatter instruction lose
    updates (SDMA CCE RMW races), so within a batch each target's k-th
    occurrence maps to a distinct accumulator row (occurrence slots);
    batches alternate between NCHAIN accumulator tensors so the WAW
    serialization chains overlap.
  - Self-loop (masked x @ W_self) initializes accumulator rows [0, nlp)
    of chain 0; all other accumulator rows are zero-filled first.
  - Host folds the chain accumulators (occurrence rows add into their
    target row) and concatenates cores.
"""

import numpy as np
import ml_dtypes

import concourse.bass as bass
import concourse.bacc as bacc
import concourse.tile as tile
import concourse.mybir as mybir
from concourse import library_config
from concourse.bass_utils import run_bass_kernel_spmd

F32 = mybir.dt.float32
BF16 = mybir.dt.bfloat16
I16 = mybir.dt.int16

NCORE = 8
R = 32            # num_relations (row R of rbw is the self-loop)
NBANK = 4
BANKSZ = 25000
CB = 7            # chunks per gather batch (transpose dma_gather caps at 896 idxs)
SPAIR = 2         # gather batches per scatter batch
NCHAIN = 2        # parallel scatter WAW chains


def host_prep(x, node_keep_mask, source, target, edge_type, edge_weights,
              bases, relation_base_weights):
    n, d = x.shape
    nl = n // NCORE
    nblk = (nl + 127) // 128
    nlp = nblk * 128
    f32 = np.float32

    W = np.einsum("rb,bdo->rdo", relation_base_weights.astype(f32),
                  bases.astype(f32)).astype(f32)          # (R+1, 128, 128)
    wsb_h = np.ascontiguousarray(
        W.transpose(1, 0, 2).reshape(d, (R + 1) * d)).astype(ml_dtypes.bfloat16)

    src2 = np.concatenate([source, target]).astype(np.int64)
    tgt2 = np.concatenate([target, source]).astype(np.int64)
    et2 = np.concatenate([edge_type, edge_type]).astype(np.int64)
    ew2 = np.concatenate([edge_weights, edge_weights]).astype(f32)

    owner = tgt2 // nl
    tloc = (tgt2 - owner * nl).astype(np.int64)
    bank = src2 // BANKSZ

    # pass 1: per-core per-(bank, rel) counts -> uniform chunk table
    binof = bank * R + et2                     # 0 .. NBANK*R-1
    cnt = np.zeros((NCORE, NBANK * R), np.int64)
    for c in range(NCORE):
        cnt[c] = np.bincount(binof[owner == c], minlength=NBANK * R)
    K = np.ceil(cnt.max(axis=0) / 128).astype(np.int64)   # chunks per bin

    chunk_rel, chunk_bank, bin_start = [], [], {}
    for bk in range(NBANK):
        for rr in range(R):
            b = bk * R + rr
            bin_start[b] = len(chunk_rel)
            chunk_rel += [rr] * int(K[b])
            chunk_bank += [bk] * int(K[b])
    NCH = len(chunk_rel)

    # gather batches: per bank run, split into <=CB chunk windows
    batches = []
    c0 = 0
    for bk in range(NBANK):
        run = int(K[bk * R:(bk + 1) * R].sum())
        for s in range(c0, c0 + run, CB):
            batches.append((s, min(s + CB, c0 + run), bk))
        c0 += run
    # scatter batches: SPAIR consecutive gather batches
    sbatches = [(batches[i][0], batches[min(i + SPAIR, len(batches)) - 1][1])
                for i in range(0, len(batches), SPAIR)]

    # pass 2: per-core edge assignment
    per_core, emaps, extra_counts = [], [], []
    for c in range(NCORE):
        m = owner == c
        sc, tc, rc, wc, bc = src2[m], tloc[m], et2[m], ew2[m], bank[m]
        gsrc = np.zeros(NCH * 128, np.int16)
        ewst = np.zeros((128, NCH), f32)
        sid_t = np.full(NCH * 128, -1, np.int64)    # target per slot, -1 pad
        binc = bc * R + rc
        order = np.lexsort((tc, binc))
        for b in range(NBANK * R):
            kk = int(K[b])
            if kk == 0:
                continue
            sel = order[np.searchsorted(binc[order], b):
                        np.searchsorted(binc[order], b + 1)]
            cs = bin_start[b]
            for i, e in enumerate(sel):
                ci, j = cs + (i % kk), i // kk
                pos = ci * 128 + j
                gsrc[pos] = sc[e] - (bc[e] * BANKSZ)
                ewst[j, ci] = wc[e]
                sid_t[pos] = tc[e]

        sid = np.full(NCH * 128, nl, np.int64)      # pads -> trash row nl
        extradicts = [dict() for _ in range(NCHAIN)]
        for bi, (b0, b1) in enumerate(sbatches):
            q = bi % NCHAIN
            occ = {}
            for pos in range(b0 * 128, b1 * 128):
                t = sid_t[pos]
                if t < 0:
                    continue
                mm = occ.get(t, 0)
                occ[t] = mm + 1
                if mm == 0:
                    sid[pos] = t
                else:
                    ed = extradicts[q]
                    sid[pos] = nlp + ed.setdefault((t, mm), len(ed))
        emap = []
        for q in range(NCHAIN):
            em = np.zeros(len(extradicts[q]), np.int64)
            for (t, _), i in extradicts[q].items():
                em[i] = t
            emap.append(em)
        emaps.append(emap)
        extra_counts.append([len(e) for e in extradicts])

        xm = (x[c * nl:(c + 1) * nl].astype(f32)
              * node_keep_mask[c * nl:(c + 1) * nl].astype(f32)[:, None])
        xmt = np.zeros((128, nlp), f32)
        xmt[:, :nl] = xm.T
        per_core.append({
            "gsrc": gsrc, "ewst": ewst, "sid": sid,
            "xmt": np.ascontiguousarray(xmt.astype(ml_dtypes.bfloat16)),
        })

    EXTRA = max(max(ec) for ec in extra_counts)
    S = nlp + ((EXTRA + 127) // 128) * 128 + 128
    assert S < 32000, S

    xb = np.ascontiguousarray(x.astype(ml_dtypes.bfloat16))

    def wrap16r(idx):
        w = idx.reshape(-1, 16).T
        return np.ascontiguousarray(np.tile(w, (8, 1)))

    for c in range(NCORE):
        pc = per_core[c]
        pc["xb"] = xb
        pc["wsb"] = wsb_h
        pc["gidx"] = wrap16r(pc.pop("gsrc"))
        pc["sidx"] = wrap16r(pc.pop("sid").astype(np.int16))
        pc["ew"] = np.ascontiguousarray(pc.pop("ewst"))

    cfg = dict(n=n, nl=nl, nblk=nblk, nlp=nlp, NCH=NCH, S=S,
               chunk_rel=tuple(chunk_rel),
               batches=tuple(batches), sbatches=tuple(sbatches))
    return per_core, cfg, emaps


def build_program(cfg):
    n = cfg["n"]
    nblk = cfg["nblk"]
    nlp = cfg["nlp"]
    NCH = cfg["NCH"]
    S = cfg["S"]
    chunk_rel = cfg["chunk_rel"]
    batches = cfg["batches"]
    sbatches = cfg["sbatches"]

    nc = bacc.Bacc(None, target_bir_lowering=False, debug=False)

    xb = nc.declare_dram_parameter("xb", [n, 128], BF16, isOutput=False)
    wsb = nc.declare_dram_parameter("wsb", [128, (R + 1) * 128], BF16,
                                    isOutput=False)
    xmt = nc.declare_dram_parameter("xmt", [128, nlp], BF16, isOutput=False)
    gidx = nc.declare_dram_parameter("gidx", [128, NCH * 8], I16, isOutput=False)
    sidx = nc.declare_dram_parameter("sidx", [128, NCH * 8], I16, isOutput=False)
    ew = nc.declare_dram_parameter("ew", [128, NCH], F32, isOutput=False)
    outq = [nc.declare_dram_parameter(f"outq{q}", [S, 128], F32, isOutput=True)
            for q in range(NCHAIN)]

    with tile.TileContext(nc) as tc:
        nc.gpsimd.load_library(library_config.mlp)
        with tc.tile_pool(name="const", bufs=1) as cst:
            wsb_t = cst.tile([128, (R + 1) * 128], BF16)
            nc.sync.dma_start(out=wsb_t[:], in_=wsb[:])
            xmt_t = cst.tile([128, nlp], BF16)
            nc.sync.dma_start(out=xmt_t[:], in_=xmt[:])
            gidx_t = cst.tile([128, NCH * 8], I16)
            nc.sync.dma_start(out=gidx_t[:], in_=gidx[:])
            sidx_t = cst.tile([128, NCH * 8], I16)
            nc.sync.dma_start(out=sidx_t[:], in_=sidx[:])
            ew_t = cst.tile([128, NCH], F32)
            nc.sync.dma_start(out=ew_t[:], in_=ew[:])
            zt = cst.tile([128, 1024], F32)
            nc.vector.memset(zt[:], 0.0)
            initbuf = cst.tile([128, nblk * 128], F32)

            # self-loop: initbuf[:, b*128+o] (partition t) = xmt_b^T @ W_self
            with tc.tile_pool(name="slps", bufs=4, space="PSUM") as slps:
                for b in range(nblk):
                    mm = slps.tile([128, 128], F32, tag="sl")
                    nc.tensor.matmul(
                        out=mm[:], lhsT=xmt_t[:, b * 128:(b + 1) * 128],
                        rhs=wsb_t[:, R * 128:(R + 1) * 128],
                        start=True, stop=True)
                    nc.scalar.copy(out=initbuf[:, b * 128:(b + 1) * 128],
                                   in_=mm[:])

            # accumulator init: chain0 rows [0,nlp) = self-loop, rest zeros
            nc.sync.dma_start(
                out=outq[0][0:nlp, :].rearrange("(b p) o -> p b o", p=128),
                in_=initbuf[:])
            for q in range(NCHAIN):
                r0 = nlp if q == 0 else 0
                while r0 < S:
                    r1 = min(r0 + 1024, S)
                    nc.sync.dma_start(out=outq[q][r0:r1, :],
                                      in_=zt[:, :r1 - r0])
                    r0 = r1

            gi = 0
            with (
                tc.tile_pool(name="gp", bufs=3) as gp,
                tc.tile_pool(name="mp", bufs=3) as mp,
                tc.tile_pool(name="psp", bufs=6, space="PSUM") as psp,
            ):
                for si_, (s0, s1) in enumerate(sbatches):
                    msg = mp.tile([128, SPAIR * CB * 128], F32, tag="msg")
                    while gi < len(batches) and batches[gi][0] < s1:
                        c0, c1, bk = batches[gi]
                        gi += 1
                        nb = c1 - c0
                        ni = nb * 128
                        xT = gp.tile([128, CB * 128], BF16, tag="xT")
                        nc.gpsimd.dma_gather(
                            xT[:, :ni].rearrange("p (c e) -> p c e", c=1),
                            xb[bk * BANKSZ:min((bk + 1) * BANKSZ, n), :],
                            gidx_t[:, c0 * 8:c1 * 8], ni, ni, 128,
                            transpose=True)
                        for i in range(nb):
                            ci = c0 + i
                            rel = chunk_rel[ci]
                            mo = (ci - s0) * 128
                            mm = psp.tile([128, 128], F32, tag="mm")
                            nc.tensor.matmul(
                                out=mm[:], lhsT=xT[:, i * 128:(i + 1) * 128],
                                rhs=wsb_t[:, rel * 128:(rel + 1) * 128],
                                start=True, stop=True)
                            if i % 2 == 0:
                                nc.vector.tensor_scalar(
                                    out=msg[:, mo:mo + 128], in0=mm[:],
                                    scalar1=ew_t[:, ci:ci + 1], scalar2=None,
                                    op0=mybir.AluOpType.mult)
                            else:
                                nc.scalar.activation(
                                    out=msg[:, mo:mo + 128], in_=mm[:],
                                    func=mybir.ActivationFunctionType.Copy,
                                    scale=ew_t[:, ci:ci + 1])
                    sn = s1 - s0
                    nc.gpsimd.dma_scatter_add(
                        outq[si_ % NCHAIN][:, :],
                        msg[:, :sn * 128].rearrange("p (c e) -> p c e", c=sn),
                        sidx_t[:, s0 * 8:s1 * 8], sn * 128, sn * 128, 128)

    nc.finalize()
    return nc


_PROGRAM_CACHE = {}


def _get_program(cfg):
    key = tuple(sorted((k, v) for k, v in cfg.items()))
    if key not in _PROGRAM_CACHE:
        _PROGRAM_CACHE[key] = build_program(cfg)
    return _PROGRAM_CACHE[key]


def kernel(x, node_keep_mask, source, target, edge_type, edge_weights,
           bases, relation_base_weights):
    per_core, cfg, emaps = host_prep(
        x, node_keep_mask, source, target, edge_type, edge_weights,
        bases, relation_base_weights)
    nc = _get_program(cfg)
    res = run_bass_kernel_spmd(nc, per_core, list(range(NCORE)))
    nl, nlp = cfg["nl"], cfg["nlp"]
    out = np.empty((cfg["n"], 128), np.float32)
    for c in range(NCORE):
        acc = None
        for q in range(NCHAIN):
            dat = np.asarray(res.results[c][f"outq{q}"], np.float32)
            o = dat[:nl].copy() if acc is None else dat[:nl]
            if acc is None:
                acc = o
            else:
                acc += o
            em = emaps[c][q]
            if len(em):
                np.add.at(acc, em, dat[nlp:nlp + len(em)])
        out[c * nl:(c + 1) * nl] = acc
    return out


# revision 7
# speedup vs baseline: 1.2763x; 1.2763x over previous
"""BasesDecomposition (R-GCN style) message passing kernel for Trainium2.

Two-phase design (8 NeuronCores, SPMD — one program, per-core data), bf16:
  - Nodes sharded by row: core c owns targets [c*nl, (c+1)*nl).
  - Edges symmetrized on host, partitioned by target-owner core.
  - Per-relation weights W_r = sum_b rbw[r, b] * bases[b] on host (bf16).
  - Phase 1 (messages): per 128-edge chunk (relation-pure, uniform group
    size G, block-sorted within relation): indirect-gather bf16 x[src]
    rows, PE-transpose, matmul with W_r, write bf16 message rows to DRAM
    md. Gathers alternate SWDGE queues to overlap ring waits.
  - Phase 2 (aggregate): per 128-target block: one wide indirect gather
    of SL-row intervals covering the block's per-relation runs
    -> [128, SL*128] bf16 tile; SL one-hot matmuls against
    host-precomputed T slices (streamed bf16 "tmat": onehot * edge
    weight), plus the self-loop matmul W_self^T @ xm^T; store out^T.
  - Host reassembles out from the per-core out^T blocks.
"""

import numpy as np
import ml_dtypes

import concourse.bass as bass
import concourse.bacc as bacc
import concourse.tile as tile
import concourse.mybir as mybir
from concourse.bass_utils import run_bass_kernel_spmd

F32 = mybir.dt.float32
BF16 = mybir.dt.bfloat16
I32 = mybir.dt.int32

NCORE = 8
R = 32  # num_relations (relation id R is the self-loop row of rbw)
SL_CANDIDATES = (8, 10, 12, 16)  # md rows per cover index in phase 2


def _ranks_within_group(keys, order, nbins):
    counts = np.bincount(keys, minlength=nbins)
    starts = np.concatenate([[0], np.cumsum(counts)[:-1]])
    r = np.empty(len(keys), np.int64)
    r[order] = np.arange(len(keys)) - starts[keys[order]]
    return r


def host_prep(x, node_keep_mask, source, target, edge_type, edge_weights,
              bases, relation_base_weights):
    n, d = x.shape
    assert n % NCORE == 0
    nl = n // NCORE
    nblk = (nl + 127) // 128
    nlp = nblk * 128

    f32 = np.float32
    bf16 = ml_dtypes.bfloat16
    W = np.einsum("rb,bdo->rdo", relation_base_weights.astype(f32),
                  bases.astype(f32)).astype(f32)  # (R+1, 128, 128)
    wsb_h = np.ascontiguousarray(
        W.transpose(1, 0, 2).reshape(d, (R + 1) * d)).astype(bf16)

    src2 = np.concatenate([source, target]).astype(np.int64)
    tgt2 = np.concatenate([target, source]).astype(np.int64)
    et2 = np.concatenate([edge_type, edge_type]).astype(np.int64)
    ew2 = np.concatenate([edge_weights, edge_weights]).astype(f32)

    owner = tgt2 // nl
    tloc = tgt2 - owner * nl
    blk = tloc // 128
    tin = (tloc - blk * 128).astype(np.int64)

    # phase-1: uniform relation-group size G across (core, relation);
    # within a relation, order edges by target block (for phase-2 runs)
    cr = owner * R + et2
    cnt_cr = np.bincount(cr, minlength=NCORE * R)
    G = int(np.ceil(max(int(cnt_cr.max()), 1) / 128)) * 128
    ep1 = R * G
    ng1 = ep1 // 128
    order1 = np.lexsort((blk, cr))
    r1 = _ranks_within_group(cr, order1, NCORE * R)
    pos1 = et2 * G + r1  # core-local md row of each edge

    # per-(core, rel, blk) run lengths and starts (within the relation group)
    crb = cr * nblk + blk
    cnt_crb = np.bincount(crb, minlength=NCORE * R * nblk).reshape(
        NCORE, R, nblk)
    run_start = np.zeros_like(cnt_crb)
    run_start[:, :, 1:] = np.cumsum(cnt_crb, axis=2)[:, :, :-1]

    for SL in SL_CANDIDATES:
        n_iv = np.ceil(cnt_crb / SL).sum(axis=1).max()
        if n_iv <= 128:
            break
    else:
        raise AssertionError(f"no SL fits: {n_iv} intervals")

    xb = np.ascontiguousarray(x.astype(bf16))

    per_core = []
    for c in range(NCORE):
        m = owner == c
        gsrc_flat = np.zeros(ep1, np.int32)
        gsrc_flat[pos1[m]] = src2[m].astype(np.int32)
        gsrc_h = np.ascontiguousarray(gsrc_flat.reshape(ng1, 128).T)

        # md row -> edge id map for this core
        edge_of_row = np.full(ep1, -1, np.int64)
        edge_ids = np.nonzero(m)[0]
        edge_of_row[pos1[edge_ids]] = edge_ids

        # phase-2 cover: per block, interval starts covering the runs
        cidx_h = np.zeros((128, nblk), np.int32)
        vlen_h = np.zeros((128, nblk), np.int64)
        for b in range(nblk):
            iv = []
            for r in range(R):
                s = r * G + int(run_start[c, r, b])
                ln = int(cnt_crb[c, r, b])
                for off in range(0, ln, SL):
                    st = min(s + off, ep1 - SL)
                    iv.append((st, min(SL, s + ln - st)))
            assert len(iv) <= 128, f"cover overflow: {len(iv)} intervals"
            for p, (st, vl) in enumerate(iv):
                cidx_h[p, b] = st
                vlen_h[p, b] = vl

        # map covered rows -> host-built T slices (tmat) in cover layout
        rows = cidx_h.astype(np.int64)[:, :, None] + np.arange(SL)
        ev = edge_of_row[rows]  # [128, nblk, SL]
        in_run = np.arange(SL)[None, None, :] < vlen_h[:, :, None]
        valid = (ev >= 0) & in_run
        evc = np.where(valid, ev, 0)
        same_blk = blk[evc] == np.arange(nblk)[None, :, None]
        use = valid & same_blk
        assert int(use.sum()) == len(edge_ids), (
            f"cover mismatch: {int(use.sum())} vs {len(edge_ids)}")
        tcol = np.where(use, tin[evc], 0)           # [128, nblk, SL]
        tscl = np.where(use, ew2[evc], 0.0).astype(f32)
        # tmat[p, b, k, t] = tscl if tcol == t else 0
        tmat = np.zeros((128, nblk, SL, 128), f32)
        pp, bb, kk = np.nonzero(use)
        tmat[pp, bb, kk, tcol[pp, bb, kk]] = tscl[pp, bb, kk]
        tmat_h = np.ascontiguousarray(
            tmat.reshape(128, nblk * SL * 128).astype(bf16))

        xm = (x[c * nl:(c + 1) * nl].astype(f32)
              * node_keep_mask[c * nl:(c + 1) * nl].astype(f32)[:, None])
        xmt_h = np.zeros((128, nlp), f32)
        xmt_h[:, :nl] = xm.T

        per_core.append({
            "xg": xb,
            "wsb": wsb_h,
            "xmt": np.ascontiguousarray(xmt_h.astype(bf16)),
            "gsrc": gsrc_h,
            "cidx": np.ascontiguousarray(cidx_h),
            "tmat": tmat_h,
        })

    cfg = dict(n=n, nl=nl, nblk=nblk, nlp=nlp, G=G, ep1=ep1, ng1=ng1, SL=SL)
    return per_core, cfg


def build_program(cfg):
    n = cfg["n"]
    nblk = cfg["nblk"]
    nlp = cfg["nlp"]
    G = cfg["G"]
    ep1 = cfg["ep1"]
    ng1 = cfg["ng1"]
    SL = cfg["SL"]

    nc = bacc.Bacc(None, target_bir_lowering=False, debug=False,
                   num_swdge_queues=2)

    xg = nc.declare_dram_parameter("xg", [n, 128], BF16, isOutput=False)
    wsb = nc.declare_dram_parameter("wsb", [128, (R + 1) * 128], BF16,
                                    isOutput=False)
    xmt = nc.declare_dram_parameter("xmt", [128, nlp], BF16, isOutput=False)
    gsrc = nc.declare_dram_parameter("gsrc", [128, ng1], I32, isOutput=False)
    cidx = nc.declare_dram_parameter("cidx", [128, nblk], I32, isOutput=False)
    tmat = nc.declare_dram_parameter("tmat", [128, nblk * SL * 128], BF16,
                                     isOutput=False)
    outT = nc.declare_dram_parameter("outT", [128, nlp], F32, isOutput=True)

    md = nc.dram_tensor("md", [ep1, 128], BF16)

    ident_d = nc.inline_tensor(np.eye(128, dtype=np.float32), name="ident_c")

    with tile.TileContext(nc) as tc:
        with tc.tile_pool(name="const", bufs=1) as constp:
            wsb_t = constp.tile([128, (R + 1) * 128], BF16)
            nc.sync.dma_start(out=wsb_t[:], in_=wsb[:])
            xmt_t = constp.tile([128, nlp], BF16)
            nc.sync.dma_start(out=xmt_t[:], in_=xmt[:])
            gsrc_t = constp.tile([128, ng1], I32)
            nc.sync.dma_start(out=gsrc_t[:], in_=gsrc[:])
            cidx_t = constp.tile([128, nblk], I32)
            nc.sync.dma_start(out=cidx_t[:], in_=cidx[:])
            ident_f = constp.tile([128, 128], F32)
            nc.sync.dma_start(out=ident_f[:], in_=ident_d[:])
            ident = constp.tile([128, 128], BF16)
            nc.vector.tensor_copy(out=ident[:], in_=ident_f[:])

            # ---------------- Phase 1: messages ----------------
            with (
                tc.tile_pool(name="p1", bufs=20) as p1,
                tc.tile_pool(name="p1ps", bufs=4, space="PSUM") as p1ps,
            ):
                for c in range(ng1):
                    xga = p1.tile([128, 128], BF16, tag="xgather")
                    ins = nc.gpsimd.indirect_dma_start(
                        out=xga[:], out_offset=None, in_=xg[:, :],
                        in_offset=bass.IndirectOffsetOnAxis(
                            ap=gsrc_t[:, c:c + 1], axis=0))
                    if c % 2:
                        ins.ins.queue = "qPoolDynamic1"
                    tp = p1ps.tile([128, 128], BF16, tag="tpsum")
                    nc.tensor.transpose(out=tp[:], in_=xga[:],
                                        identity=ident[:])
                    xT = p1.tile([128, 128], BF16, tag="xT")
                    nc.vector.tensor_copy(out=xT[:], in_=tp[:])
                    mp = p1ps.tile([128, 128], F32, tag="mpsum")
                    r = (c * 128) // G
                    nc.tensor.matmul(
                        out=mp[:], lhsT=xT[:],
                        rhs=wsb_t[:, 128 * r:128 * (r + 1)],
                        start=True, stop=True)
                    ms = p1.tile([128, 128], BF16, tag="mstage")
                    nc.scalar.copy(out=ms[:], in_=mp[:])
                    nc.sync.dma_start(out=md[128 * c:128 * (c + 1), :],
                                      in_=ms[:])

            # ---------------- Phase 2: aggregate ----------------
            with (
                tc.tile_pool(name="p2", bufs=8) as p2,
                tc.tile_pool(name="p2ps", bufs=4, space="PSUM") as p2ps,
            ):
                for b in range(nblk):
                    mg = p2.tile([128, SL * 128], BF16, tag="mg")
                    ins = nc.gpsimd.indirect_dma_start(
                        out=mg[:], out_offset=None, in_=md[:, :],
                        in_offset=bass.IndirectOffsetOnAxis(
                            ap=cidx_t[:, b:b + 1], axis=0))
                    if b % 2:
                        ins.ins.queue = "qPoolDynamic1"
                    tt = p2.tile([128, SL * 128], BF16, tag="T")
                    nc.sync.dma_start(
                        out=tt[:],
                        in_=tmat[:, b * SL * 128:(b + 1) * SL * 128])
                    ps = p2ps.tile([128, 128], F32, tag="acc")
                    for j in range(SL):
                        nc.tensor.matmul(
                            out=ps[:],
                            lhsT=mg[:, 128 * j:128 * (j + 1)],
                            rhs=tt[:, 128 * j:128 * (j + 1)],
                            start=(j == 0), stop=False)
                    nc.tensor.matmul(
                        out=ps[:],
                        lhsT=wsb_t[:, R * 128:(R + 1) * 128],
                        rhs=xmt_t[:, 128 * b:128 * (b + 1)],
                        start=False, stop=True)
                    ob = p2.tile([128, 128], F32, tag="ob")
                    nc.vector.tensor_copy(out=ob[:], in_=ps[:])
                    nc.sync.dma_start(out=outT[:, 128 * b:128 * (b + 1)],
                                      in_=ob[:])

    nc.finalize()
    return nc


_PROGRAM_CACHE = {}


def _get_program(cfg):
    key = tuple(sorted(cfg.items()))
    if key not in _PROGRAM_CACHE:
        _PROGRAM_CACHE[key] = build_program(cfg)
    return _PROGRAM_CACHE[key]


def kernel(x, node_keep_mask, source, target, edge_type, edge_weights,
           bases, relation_base_weights):
    per_core, cfg = host_prep(x, node_keep_mask, source, target, edge_type,
                              edge_weights, bases, relation_base_weights)
    nc = _get_program(cfg)
    res = run_bass_kernel_spmd(nc, per_core, list(range(NCORE)))
    nl = cfg["nl"]
    out = np.empty((cfg["n"], 128), np.float32)
    for c in range(NCORE):
        out[c * nl:(c + 1) * nl] = \
            np.asarray(res.results[c]["outT"], np.float32)[:, :nl].T
    return out
